# revision 10
# baseline (speedup 1.0000x reference)
"""Trainium2 Bass kernel for CompanySpecificHeads (MoE-style routed MLP heads).

Semantics (matching the reference):
    out[b] = gelu(z[b] @ W1[cid[b]] + b1[cid[b]]) @ W2[cid[b]] + b2[cid[b]]

Strategy: expert-parallel across 8 NeuronCores. Companies are sharded
8-per-core; tokens are routed (gathered by company) to their company's core
on the host, padded to a fixed per-company capacity, and each core runs a
grouped GEMM -> gelu -> dot pipeline over its 8 companies:

  Layer 1 (per company c, h on partitions):
      psum[h, t] = sum_d W1[c][d, h] * zT[c][d, t]      (PE, fp16 operands)
  Gelu: ACT engine, PSUM -> SBUF (fp16 out), with b1 folded in via the
      per-partition bias operand of the activation instruction.
  Layer 2: psum2[1, t] += W2[c][hj]^T @ gelu_h[hj, t]   (8 K=128 matmuls)

Host does the unshard/scatter back to [B, 1] and adds b2 (exact, fp32).

Schedule notes (from trace analysis of the previous version):
  - The w1 stream is the critical path (8MB/core at ~341GB/s ~= 24.6us), so
    its first DMA trigger is the first instruction on the sync ring after
    the framework preamble; small loads (b1/w2/z) ride the scalar ring.
  - PE warmup matmuls depend only on a vector-engine memset so they start
    right after the preamble; the HAM clock gate then un-throttles the PE
    (1.2 -> 2.4 GHz) before the first real matmul instead of 12us in.
  - Layer-2 matmuls for group (c,g) are emitted after layer-1 of the next
    group so the PE never stalls waiting on the ACT engine's gelu.
  - Outputs are staged in SBUF and stored with HWDGE (sync ring) in two
    chunks: companies 0-6 overlap company 7's compute; the final store
    carries only company 7 (sub-us tail).
"""

import numpy as np

B, C, D, H = 4096, 64, 512, 1024
NCORES = 8
CPC = C // NCORES  # companies per core
KC = D // 128      # contraction chunks of 128
HC = H // 128      # h chunks of 128

_COMPILED = {}


def _build(TW, NTT, dtype_name):
    """Build the Bass/Tile program for per-company token capacity NTT*TW."""
    import concourse.bass as bass
    import concourse.bacc as bacc
    import concourse.mybir as mybir
    from concourse.tile import TileContext
    from contextlib import ExitStack

    f32 = mybir.dt.float32
    dt_op = getattr(mybir.dt, dtype_name)

    nc = bacc.Bacc(None, target_bir_lowering=False)

    # zt is stored partition-major so one DMA moves it with large packets.
    zt_d = nc.dram_tensor("zt", [128, CPC, NTT, KC, TW], dt_op, kind="ExternalInput")
    # w1 stored as [c][p][g][k][h-half]: a whole company loads linearly
    # with 8KB contiguous per partition (full-rate packets).
    w1_d = nc.dram_tensor(
        "w1", [CPC, 128, 2, KC, H // 2], dt_op, kind="ExternalInput"
    )
    # b1 as columns: b1c[m, (c*2+g)*KC + j] = b1[c][512g+128j+m], fp32.
    b1_d = nc.dram_tensor("b1c", [128, CPC * 2 * KC], f32, kind="ExternalInput")
    w2_d = nc.dram_tensor("w2h", [128, CPC * HC], dt_op, kind="ExternalInput")
    out_d = nc.dram_tensor("out", [1, CPC * NTT * TW], f32, kind="ExternalOutput")

    gelu = mybir.ActivationFunctionType.Gelu

    with TileContext(nc) as tc, ExitStack() as ctx:
        const = ctx.enter_context(tc.tile_pool(name="const", bufs=1))

        # Per-company weights on the SP HWDGE ring, one company per DMA,
        # issued before anything else so the stream starts as early as
        # possible. The ring drains FIFO at full bandwidth and compute
        # pipelines behind the weight stream.
        w1p = ctx.enter_context(tc.tile_pool(name="w1p", bufs=1))
        w1ts = []
        for c in range(CPC):
            w1t = w1p.tile([128, 2, KC, H // 2], dt_op, name=f"w1_{c}")
            if c in (0, CPC - 1):
                # First/last company split into g-halves: company 0's
                # layer-1 g0 starts half a DMA earlier (head), and company
                # 7's g1 compute is all that trails the stream (tail).
                nc.sync.dma_start(out=w1t[:, 0], in_=w1_d[c, :, 0])
                nc.sync.dma_start(out=w1t[:, 1], in_=w1_d[c, :, 1])
            else:
                nc.sync.dma_start(out=w1t[:], in_=w1_d[c])
            w1ts.append(w1t)

        # Small constants + routed tokens on the ACT HWDGE ring: its
        # dispatch overlaps the SP ring's w1 dispatches. Tokens are split
        # so company c's slice lands (well) before w1[c] does — the first
        # bytes out of an idle DMA system take ~5us, so the early
        # companies' tokens go in small DMAs ordered by need.
        zall = const.tile([128, CPC, NTT, KC, TW], dt_op)
        b1t = const.tile([128, CPC * 2 * KC], f32)
        w2t = const.tile([128, CPC * HC], dt_op)
        nc.scalar.dma_start(out=zall[:, 0:1], in_=zt_d[:, 0:1])
        nc.scalar.dma_start(out=zall[:, 1:2], in_=zt_d[:, 1:2])
        nc.scalar.dma_start(out=b1t[:], in_=b1_d[:])
        nc.scalar.dma_start(out=w2t[:], in_=w2_d[:])
        nc.scalar.dma_start(out=zall[:, 2:4], in_=zt_d[:, 2:4])
        nc.scalar.dma_start(out=zall[:, 4:6], in_=zt_d[:, 4:6])
        nc.scalar.dma_start(out=zall[:, 6:8], in_=zt_d[:, 6:8])

        # Staged per-company outputs; two HWDGE stores at the end.
        oall = const.tile([1, CPC * NTT * TW], f32)

        hp = ctx.enter_context(tc.tile_pool(name="hp", bufs=4))
        pp = ctx.enter_context(tc.tile_pool(name="pp", bufs=4, space="PSUM"))
        opp = ctx.enter_context(tc.tile_pool(name="opp", bufs=2, space="PSUM"))

        # PE warmup: dependency-free matmuls on scratch data so the HAM
        # clock gate un-throttles the PE (1.2 -> 2.4 GHz takes ~3.4us of
        # sustained activity) while the first w1 DMA streams in. The
        # scratch memset runs on the otherwise-idle vector engine so the
        # warmup starts right after the framework preamble.
        wsc = const.tile([128, 384], dt_op)
        nc.vector.memset(wsc[:], 0.0)
        wps = ctx.enter_context(tc.tile_pool(name="wps", bufs=1, space="PSUM"))
        wp = wps.tile([128, 384], f32)
        for _ in range(10):
            nc.tensor.matmul(wp[:], wsc[:, :128], wsc[:], start=True, stop=True)

        # Software-pipelined job list: layer-2 of job i is emitted after
        # layer-1 of job i+1, so the PE keeps streaming layer-1 matmuls
        # while the ACT engine computes job i's gelu.
        jobs = [(c, tt, g) for c in range(CPC) for tt in range(NTT) for g in range(2)]
        pending = []  # (c, tt, g, ps_or_ht state)
        osums = {}

        def emit_l1(c, tt, g):
            ps = pp.tile([128, KC * TW], f32)
            for j in range(KC):
                for k in range(KC):
                    nc.tensor.matmul(
                        ps[:, j * TW:(j + 1) * TW],
                        w1ts[c][:, g, k, 128 * j:128 * (j + 1)],
                        zall[:, c, tt, k, :],
                        start=(k == 0),
                        stop=(k == KC - 1),
                    )
            # Bias on the (otherwise idle) vector engine: ONE in-place
            # broadcast add into PSUM per group. A stride-0 AP repeats
            # each b1 value across the token axis, so the whole group is
            # a single DVE instruction — per-instruction fixed cost
            # (~0.4us) on DVE/ACT is what paced the previous versions.
            idx = (c * 2 + g) * KC
            bias_b = b1t[:, idx:idx + KC].unsqueeze(2).broadcast_to([128, KC, TW])
            psv = ps[:].rearrange("p (j t) -> p j t", j=KC)
            nc.vector.tensor_add(psv, psv, bias_b)
            ht = hp.tile([128, KC * TW], dt_op)
            nc.scalar.activation(ht[:], ps[:], gelu)
            return ht

        def emit_l2(c, tt, g, ht):
            if g == 0:
                osums[(c, tt)] = opp.tile([1, TW], f32, name="osum")
            osum = osums[(c, tt)]
            for j in range(KC):
                jj = KC * g + j
                nc.tensor.matmul(
                    osum[:],
                    w2t[:, HC * c + jj:HC * c + jj + 1],
                    ht[:, j * TW:(j + 1) * TW],
                    start=(jj == 0),
                    stop=(jj == HC - 1),
                )
            if g == 1:
                off = (c * NTT + tt) * TW
                nc.vector.tensor_copy(oall[:, off:off + TW], osum[:])
                del osums[(c, tt)]

        for job in jobs:
            ht = emit_l1(*job)
            if pending:
                emit_l2(*pending.pop(0))
            pending.append((*job, ht))
        while pending:
            emit_l2(*pending.pop(0))

        # Stores on the sync ring (drained long before): companies 0-6
        # fire while company 7 computes; the final store is tiny.
        osplit = (CPC - 1) * NTT * TW
        nc.sync.dma_start(out=out_d[:, :osplit], in_=oall[:, :osplit])
        nc.sync.dma_start(out=out_d[:, osplit:], in_=oall[:, osplit:])

    nc.finalize()
    return nc


def _get_compiled(TW, NTT, dtype_name):
    key = (TW, NTT, dtype_name)
    if key not in _COMPILED:
        _COMPILED[key] = _build(TW, NTT, dtype_name)
    return _COMPILED[key]


def kernel(z, company_id, W1, b1, W2, b2):
    from concourse.bass_utils import run_bass_kernel_spmd

    z = np.asarray(z, dtype=np.float32)
    cid = np.asarray(company_id).astype(np.int64).ravel()
    W1 = np.asarray(W1, dtype=np.float32)
    b1 = np.asarray(b1, dtype=np.float32)
    W2 = np.asarray(W2, dtype=np.float32)
    b2 = np.asarray(b2, dtype=np.float32)
    O = W2.shape[2]

    np_op = np.float16
    dtype_name = "float16"

    idx_by_company = [np.nonzero(cid == gc)[0] for gc in range(C)]
    max_cnt = max((len(ix) for ix in idx_by_company), default=1)
    max_cnt = max(max_cnt, 1)
    if max_cnt <= 128:
        NTT = 1
        TW = ((max_cnt + 15) // 16) * 16
    else:
        NTT = (max_cnt + 127) // 128
        TW = 128
    CAP = NTT * TW

    nc = _get_compiled(TW, NTT, dtype_name)

    in_maps = []
    for core in range(NCORES):
        # zt[p, c, tt, k, t] = z[token, 128k+p]  (partition-major)
        zt = np.zeros((128, CPC, NTT, KC, TW), dtype=np_op)
        for ci in range(CPC):
            gc = core * CPC + ci
            ix = idx_by_company[gc]
            if len(ix) == 0:
                continue
            zpad = np.zeros((CAP, D), dtype=np_op)
            zpad[: len(ix)] = z[ix].astype(np_op)
            zt[:, ci] = zpad.reshape(NTT, TW, KC, 128).transpose(3, 0, 2, 1)
        # w1[c, p, g, k, hh] = W1[gc, 128k+p, 512g+hh]
        w1 = (
            W1[core * CPC:(core + 1) * CPC]
            .reshape(CPC, KC, 128, 2, H // 2)
            .transpose(0, 2, 3, 1, 4)
            .astype(np_op)
        )
        # b1c[m, (c*2+g)*KC + j] = b1[gc, 512g+128j+m]  (fp32 columns)
        b1c = (
            b1[core * CPC:(core + 1) * CPC]
            .reshape(CPC, 2, KC, 128)
            .transpose(3, 0, 1, 2)
            .reshape(128, CPC * 2 * KC)
            .astype(np.float32)
        )
        # w2h[p, HC*c + j] = W2[gc, 128j+p, 0]
        w2h = (
            W2[core * CPC:(core + 1) * CPC, :, 0]
            .reshape(CPC, HC, 128)
            .transpose(2, 0, 1)
            .reshape(128, CPC * HC)
            .astype(np_op)
        )
        in_maps.append(
            {
                "zt": np.ascontiguousarray(zt),
                "w1": np.ascontiguousarray(w1),
                "b1c": np.ascontiguousarray(b1c),
                "w2h": np.ascontiguousarray(w2h),
            }
        )

    res = run_bass_kernel_spmd(nc, in_maps, list(range(NCORES)))

    out = np.zeros((B, O), dtype=np.float32)
    for core in range(NCORES):
        core_out = res.results[core]["out"].reshape(CPC, NTT * TW)
        for ci in range(CPC):
            gc = core * CPC + ci
            ix = idx_by_company[gc]
            if len(ix) == 0:
                continue
            out[ix, 0] = core_out[ci, : len(ix)] + b2[gc, 0]
    return out
